# revision 5
# baseline (speedup 1.0000x reference)
"""Trainium2 Bass kernel for nn_MaxMinAgg.

Computes, for full inputs m [1024, 256] f32 and weight [256, 512] f32:
    z[b, j]  = max_k min(m[b, k], weight[k, j])          (tropical max-min matmul)
    out[b,o] = max_a z[b, 4*o + a]                       (max-pool over AGG=4 groups)

Key identity: max_a min(x, w_a) = min(x, max_a w_a), so the AGG max-pool folds
into the weight: wmax[k, o] = max_a weight[k, 4o+a], and
    out[b, o] = max_k min(m[b, k], wmax[k, o])
which is 4x less elementwise work.

Distribution: data-parallel over batch across 8 NeuronCores (128 rows each =
exactly the 128 SBUF partitions); weight replicated.

Per-core algorithm:
  - DVE tensor_reduce folds weight -> wmax [k, o], PE-transpose -> wmaxT [o, k].
  - For each output column o: PE broadcasts wmaxT[o, :] across all 128
    partitions via a K=1 matmul with a ones vector (exact: 1.0*x in fp32),
    landing in PSUM; then one fused DVE tensor_tensor_reduce
    (out = min(m, bcast), accum_out = max-reduce) produces out[:, o]
    in a single pass over the data.
"""

import sys

import numpy as np

if "/opt/trn_rl_repo" not in sys.path:
    sys.path.insert(0, "/opt/trn_rl_repo")

B, IN_F, OUT_F, AGG = 1024, 256, 128, 4
N_CORES = 8
B_SH = B // N_CORES  # 128

_CACHE = {}


def emit_core_program(tc, o_d, m_d, w_d):
    """Emit the per-core Tile program.

    o_d: DRAM out [B_SH, OUT_F] f32, m_d: DRAM in [B_SH, IN_F] f32,
    w_d: DRAM in [IN_F, OUT_F*AGG] f32.
    """
    from contextlib import ExitStack

    from concourse import mybir

    nc = tc.nc
    f32 = mybir.dt.float32
    AX = mybir.AxisListType
    OP = mybir.AluOpType

    with ExitStack() as ctx:
        const = ctx.enter_context(tc.tile_pool(name="const", bufs=1))
        psum = ctx.enter_context(tc.tile_pool(name="psum_bc", bufs=2, space="PSUM"))

        m_sb = const.tile([B_SH, IN_F], f32)
        nc.sync.dma_start(out=m_sb, in_=m_d)

        # weight [256, 512] -> SBUF [128(k_p), 2(h), 512(j)], k = h*128 + k_p
        w_sb = const.tile([128, 2, OUT_F * AGG], f32)
        nc.sync.dma_start(out=w_sb, in_=w_d.rearrange("(h p) j -> p h j", p=128))

        # Fold AGG: wmax[k, o] = max_a w[k, 4o+a]
        wmax_sb = const.tile([128, 2, OUT_F], f32)
        for h in range(2):
            nc.vector.tensor_reduce(
                out=wmax_sb[:, h, :],
                in_=w_sb[:, h, :].rearrange("p (o a) -> p o a", a=AGG),
                axis=AX.X,
                op=OP.max,
            )

        # Round-trip wmax through DRAM to lay it k-major on a single
        # partition: matmul rhs operands must start at base partition 0, so
        # the per-k weight rows live in the free dim of partition 0.
        # scratch_d[h, kp, o] => flat index k*OUT_F + o with k = h*128 + kp.
        scratch_d = nc.dram_tensor(
            "w_scratch", [2, 128, OUT_F], f32, kind="Internal"
        ).ap()
        nc.sync.dma_start(
            out=scratch_d.rearrange("h kp o -> kp h o"), in_=wmax_sb
        )
        wflat = const.tile([1, IN_F * OUT_F], f32)
        nc.sync.dma_start(out=wflat, in_=scratch_d)

        ones_sb = const.tile([1, 128], f32)
        nc.vector.memset(ones_sb, 1.0)

        acc = const.tile([B_SH, OUT_F], f32)
        nc.vector.memset(acc, -3.0e38)

        # k-slots per PSUM tile (4 matmuls of 512) and per ACT copy chunk.
        KC = 16  # k per chunk
        for kg in range(IN_F // KC):
            bc = psum.tile([128, KC * OUT_F], f32, tag="bc")
            for q in range(KC * OUT_F // 512):
                nc.tensor.matmul(
                    bc[:, q * 512 : (q + 1) * 512],
                    ones_sb,
                    wflat[:, kg * KC * OUT_F + q * 512 : kg * KC * OUT_F + (q + 1) * 512],
                    start=True,
                    stop=True,
                )
            wrep = const.tile([128, KC, OUT_F], f32, tag="wrep", bufs=3)
            nc.scalar.copy(out=wrep, in_=bc)
            for i in range(KC):
                k = kg * KC + i
                # acc = max(min(wrep_k, m[:, k]), acc)  -- one fused DVE op
                nc.vector.scalar_tensor_tensor(
                    out=acc,
                    in0=wrep[:, i, :],
                    scalar=m_sb[:, k : k + 1],
                    in1=acc,
                    op0=OP.min,
                    op1=OP.max,
                )

        nc.sync.dma_start(out=o_d, in_=acc)


def _build():
    if "nc" in _CACHE:
        return _CACHE["nc"]
    import concourse.bacc as bacc
    import concourse.tile as tile
    from concourse import mybir

    f32 = mybir.dt.float32
    nc = bacc.Bacc(
        "TRN2",
        target_bir_lowering=False,
        debug=False,
        enable_asserts=True,
        num_devices=N_CORES,
    )
    m_d = nc.dram_tensor("m0", [B_SH, IN_F], f32, kind="ExternalInput").ap()
    w_d = nc.dram_tensor("w0", [IN_F, OUT_F * AGG], f32, kind="ExternalInput").ap()
    o_d = nc.dram_tensor("out0", [B_SH, OUT_F], f32, kind="ExternalOutput").ap()
    with tile.TileContext(nc) as tc:
        emit_core_program(tc, o_d, m_d, w_d)
    nc.compile()
    _CACHE["nc"] = nc
    return nc


def run(m, weight, trace=False, **spmd_kwargs):
    """Run on 8 NeuronCores; returns (full_output, BassKernelResults)."""
    from concourse.bass_utils import run_bass_kernel_spmd

    nc = _build()
    m = np.ascontiguousarray(np.asarray(m, dtype=np.float32))
    weight = np.ascontiguousarray(np.asarray(weight, dtype=np.float32))
    assert m.shape == (B, IN_F) and weight.shape == (IN_F, OUT_F * AGG)
    in_maps = [
        {"m0": m[i * B_SH : (i + 1) * B_SH], "w0": weight} for i in range(N_CORES)
    ]
    res = run_bass_kernel_spmd(
        nc, in_maps, core_ids=list(range(N_CORES)), trace=trace, **spmd_kwargs
    )
    out = np.concatenate([res.results[i]["out0"] for i in range(N_CORES)], axis=0)
    return out, res


def kernel(m, weight, agg_features=AGG, **_ignored):
    assert int(agg_features) == AGG
    out, _ = run(m, weight, trace=False)
    return out.astype(np.float32)


# revision 6
# speedup vs baseline: 1.3227x; 1.3227x over previous
"""Trainium2 Bass kernel for nn_MaxMinAgg.

Computes, for full inputs m [1024, 256] f32 and weight [256, 512] f32:
    z[b, j]  = max_k min(m[b, k], weight[k, j])          (tropical max-min matmul)
    out[b,o] = max_a z[b, 4*o + a]                       (max-pool over AGG=4 groups)

Key identity: max_a min(x, w_a) = min(x, max_a w_a), so the AGG max-pool folds
into the weight: wmax[k, o] = max_a weight[k, 4o+a], and
    out[b, o] = max_k min(m[b, k], wmax[k, o])
which is 4x less elementwise work.  All ops are exact f32 selections.

Distribution: data-parallel over batch across 8 NeuronCores (128 rows each);
weight replicated.

Per-core algorithm (partition dim = o, the 128 output features):
  - DMA broadcasts m rows from DRAM across all 128 partitions in b-chunks
    (starts immediately, no dependencies; hidden under compute).
  - weight -> wmax (DVE segmented max-reduce) -> PE transpose -> wmaxT [o, k]
    in SBUF (tiny, off critical path).
  - Per b-chunk: one big DVE tensor_tensor min (wmaxT free-broadcast over b vs
    the replicated m chunk, in place), then one DVE segmented tensor_reduce max
    over k -> outT[o, b_chunk].  Two 1x passes over the data - the DVE floor
    given this toolchain (tensor_tensor_reduce crashes the runtime; GPSIMD has
    no tensor_tensor; scan is 2 cyc/elem).
  - Final PE transpose outT -> out [b, o], DMA out.
"""

import sys

import numpy as np

if "/opt/trn_rl_repo" not in sys.path:
    sys.path.insert(0, "/opt/trn_rl_repo")

B, IN_F, OUT_F, AGG = 1024, 256, 128, 4
N_CORES = 8
B_SH = B // N_CORES  # 128

# b-chunk ramp: small first chunk so the first TT starts early while later
# chunk DMAs (4MB each) hide under compute.
B_CHUNKS = [8, 24, 32, 32, 32]

_CACHE = {}


def emit_core_program(tc, o_d, m_d, w_d):
    """Emit the per-core Tile program.

    o_d: DRAM out [B_SH, OUT_F] f32, m_d: DRAM in [B_SH, IN_F] f32,
    w_d: DRAM in [IN_F, OUT_F*AGG] f32.
    """
    from contextlib import ExitStack

    import concourse.bass as bass
    from concourse import mybir
    from concourse.masks import make_identity

    nc = tc.nc
    f32 = mybir.dt.float32
    AX = mybir.AxisListType
    OP = mybir.AluOpType

    with ExitStack() as ctx:
        const = ctx.enter_context(tc.tile_pool(name="const", bufs=1))
        mpool = ctx.enter_context(tc.tile_pool(name="mpool", bufs=2))
        psum = ctx.enter_context(tc.tile_pool(name="psum", bufs=2, space="PSUM"))

        # --- m replication chunks: DMA-broadcast from DRAM, no deps -------
        mreps = []
        b0 = 0
        for ci, bc in enumerate(B_CHUNKS):
            mrep = mpool.tile([128, 32, IN_F], f32, tag="mrep", name=f"mrep{ci}")
            src = m_d[b0 : b0 + bc, :]
            src_b = bass.AP(
                tensor=src.tensor,
                offset=src.offset,
                ap=[[0, 128]] + [list(x) for x in src.ap],
            )
            nc.sync.dma_start(out=mrep[:, :bc, :], in_=src_b)
            mreps.append(mrep)
            b0 += bc

        # --- weight -> wmax -> wmaxT [o, k] (k = h*128 + kp) --------------
        w_sb = const.tile([128, 2, OUT_F * AGG], f32)
        nc.sync.dma_start(out=w_sb, in_=w_d.rearrange("(h p) j -> p h j", p=128))
        wmax_sb = const.tile([128, 2, OUT_F], f32)
        for h in range(2):
            nc.vector.tensor_reduce(
                out=wmax_sb[:, h, :],
                in_=w_sb[:, h, :].rearrange("p (o a) -> p o a", a=AGG),
                axis=AX.X,
                op=OP.max,
            )
        ident = const.tile([128, 128], f32)
        make_identity(nc, ident)
        wmaxT = const.tile([128, 2, 128], f32)  # [o, h, kp]
        for h in range(2):
            pt = psum.tile([128, 128], f32, tag="tp")
            nc.tensor.transpose(pt, wmax_sb[:, h, :], ident)
            nc.scalar.copy(out=wmaxT[:, h, :], in_=pt)
        wmaxT_flat = wmaxT.rearrange("o h kp -> o (h kp)")  # [128, 256]

        # --- main loop: per b-chunk fused min + segmented max-reduce ------
        outT = const.tile([128, B_SH], f32)  # [o, b]
        b0 = 0
        for ci, bc in enumerate(B_CHUNKS):
            mrep = mreps[ci]
            w_bcast = wmaxT_flat.rearrange("o k -> o () k").broadcast_to(
                (128, bc, IN_F)
            )
            nc.vector.tensor_tensor(
                out=mrep[:, :bc, :], in0=w_bcast, in1=mrep[:, :bc, :], op=OP.min
            )
            nc.vector.tensor_reduce(
                out=outT[:, b0 : b0 + bc],
                in_=mrep[:, :bc, :],
                axis=AX.X,
                op=OP.max,
            )
            b0 += bc

        # --- transpose outT -> out [b, o], DMA out ------------------------
        pt_out = psum.tile([128, 128], f32, tag="tp")
        nc.tensor.transpose(pt_out, outT, ident)
        out_sb = const.tile([B_SH, OUT_F], f32)
        nc.scalar.copy(out=out_sb, in_=pt_out)
        nc.sync.dma_start(out=o_d, in_=out_sb)


def _build():
    if "nc" in _CACHE:
        return _CACHE["nc"]
    import concourse.bacc as bacc
    import concourse.tile as tile
    from concourse import mybir

    f32 = mybir.dt.float32
    nc = bacc.Bacc(
        "TRN2",
        target_bir_lowering=False,
        debug=False,
        enable_asserts=True,
        num_devices=N_CORES,
    )
    m_d = nc.dram_tensor("m0", [B_SH, IN_F], f32, kind="ExternalInput").ap()
    w_d = nc.dram_tensor("w0", [IN_F, OUT_F * AGG], f32, kind="ExternalInput").ap()
    o_d = nc.dram_tensor("out0", [B_SH, OUT_F], f32, kind="ExternalOutput").ap()
    with tile.TileContext(nc) as tc:
        emit_core_program(tc, o_d, m_d, w_d)
    nc.compile()
    _CACHE["nc"] = nc
    return nc


def run(m, weight, trace=False, **spmd_kwargs):
    """Run on 8 NeuronCores; returns (full_output, BassKernelResults)."""
    from concourse.bass_utils import run_bass_kernel_spmd

    nc = _build()
    m = np.ascontiguousarray(np.asarray(m, dtype=np.float32))
    weight = np.ascontiguousarray(np.asarray(weight, dtype=np.float32))
    assert m.shape == (B, IN_F) and weight.shape == (IN_F, OUT_F * AGG)
    in_maps = [
        {"m0": m[i * B_SH : (i + 1) * B_SH], "w0": weight} for i in range(N_CORES)
    ]
    res = run_bass_kernel_spmd(
        nc, in_maps, core_ids=list(range(N_CORES)), trace=trace, **spmd_kwargs
    )
    out = np.concatenate([res.results[i]["out0"] for i in range(N_CORES)], axis=0)
    return out, res


def kernel(m, weight, agg_features=AGG, **_ignored):
    assert int(agg_features) == AGG
    out, _ = run(m, weight, trace=False)
    return out.astype(np.float32)
